# revision 1
# baseline (speedup 1.0000x reference)
"""EGIN (GIN with edge features) forward pass on 8 Trainium2 NeuronCores.

Sharding: nodes partitioned across 8 cores (padded local shards); params
replicated; edges live on the core owning their dst node so the scatter-sum
is core-local. Per layer the full (fp32, 512B-row) node-feature table h is
AllGathered into every core's HBM and h[src] is fetched with the SWDGE
dma_gather (int16 indices over 4 source windows; fp32 rows because gather
throughput is descriptor-bound and 512B descs move 2x the bytes of 256B at
similar desc rate). Messages m = relu(h_src + bond_emb) are built on the
tensor engine (batched identity matmul for h_src + one-hot-18 matmul per
subtile for the bond embedding, accumulating in PSUM); the scatter-sum is
one-hot P^T matmuls accumulated in per-tile PSUM banks (feature-major agg;
one accumulation group per bank — a group start invalidates the whole bank).
The atom encoder is one-hot matmuls (no gather). The MLP runs feature-major;
batchnorm stats are per-core partial sums AllReduced across cores
((1+eps)/b1/b2 folded host-side where they cancel). Mean-pooling over graphs
is one-hot graph matmuls (dma_scatter_add does NOT accumulate on this HW) +
an AllReduce.

Host-side work is index marshalling / parameter repacking only.
"""

import numpy as np
import ml_dtypes

import concourse.bacc as bacc
import concourse.bass as bass
import concourse.tile as tile
import concourse.mybir as mybir
from concourse import library_config
from concourse.bass_utils import run_bass_kernel_spmd

BF16 = ml_dtypes.bfloat16
F32 = np.float32

# ---------------------------------------------------------------- config ----


class Cfg:
    def __init__(self, N=100000, E=1600000, D=128, L=3, G=1024,
                 ATOM_V=100, BOND_V=6, OUT=128, NC=8, BN_EPS=1e-5):
        assert D == 128 and OUT == 128
        assert N % NC == 0
        self.N, self.E, self.D, self.L, self.G = N, E, D, L, G
        self.ATOM_V, self.BOND_V, self.OUT, self.NC = ATOM_V, BOND_V, OUT, NC
        self.BN_EPS = BN_EPS
        self.NLOC = N // NC
        self.NPAD = ((self.NLOC + 127) // 128) * 128
        self.NT = self.NPAD // 128
        self.NPT = self.NPAD * NC
        self.NCHUNK = max(1, -(-self.NPT // 32768))
        while self.NPT % self.NCHUNK:
            self.NCHUNK += 1
        self.CH = self.NPT // self.NCHUNK
        assert self.CH <= 32768
        self.GT = min(5, self.NT)
        self.groups = []
        t0 = 0
        while t0 < self.NT:
            g = min(self.GT, self.NT - t0)
            self.groups.append((t0, g))
            t0 += g
        self.GTA = min(3, self.NT)       # atom-phase tiles per gather call
        self.agroups = []
        t0 = 0
        while t0 < self.NT:
            g = min(self.GTA, self.NT - t0)
            self.agroups.append((t0, g))
            t0 += g
        assert G % 128 == 0
        self.NGT = G // 128
        self.PADG = G + 128              # pooled table rows (+trash block)


def _wrap16(flat):
    """int16 flat idx array -> [128, n/16] wrapped layout for dma_gather."""
    n = flat.shape[0]
    assert n % 16 == 0
    w = flat.reshape(n // 16, 16).T            # idx j at [j%16, j//16]
    return np.tile(w, (8, 1)).astype(np.int16)


# ---------------------------------------------------------- preprocessing ----


def preprocess(cfg, inputs):
    """Build per-core input maps (index marshalling + param repacking only)."""
    c = cfg
    x = np.asarray(inputs["x"], np.int64)
    ex = np.asarray(inputs["ex"], np.int64)
    src = np.asarray(inputs["src"], np.int64)
    dst = np.asarray(inputs["dst"], np.int64)
    node_graph = np.asarray(inputs["node_graph"], np.int64)
    atom_emb = np.asarray(inputs["atom_emb"], F32)
    bond_emb = np.asarray(inputs["bond_emb"], F32)
    eps = np.asarray(inputs["eps"], F32)
    W1 = np.asarray(inputs["W1"], F32)
    g1 = np.asarray(inputs["g1"], F32)
    be1 = np.asarray(inputs["be1"], F32)
    W2 = np.asarray(inputs["W2"], F32)
    b2 = np.asarray(inputs["b2"], F32)
    bn_g = np.asarray(inputs["bn_g"], F32)
    bn_b = np.asarray(inputs["bn_b"], F32)
    outW = np.asarray(inputs["outW"], F32)
    outb = np.asarray(inputs["outb"], F32)

    src_pid = (src // c.NLOC) * c.NPAD + (src % c.NLOC)
    e_core = dst // c.NLOC
    dst_loc = dst % c.NLOC

    tile_of = dst_loc // 128
    chunk_of = src_pid // c.CH
    key = (e_core * c.NT + tile_of) * c.NCHUNK + chunk_of
    counts = np.bincount(key, minlength=c.NC * c.NT * c.NCHUNK)
    K_tc = max(128, int(-(-counts.max() // 128) * 128))
    KT = K_tc // 128
    TOT = c.NT * c.NCHUNK * K_tc

    # slot base per (tile, chunk): call order is (group, chunk, tile-in-group)
    base = np.zeros((c.NT, c.NCHUNK), np.int64)
    off = 0
    for (t0, gt) in c.groups:
        for ch in range(c.NCHUNK):
            for tl in range(gt):
                base[t0 + tl, ch] = off
                off += K_tc
    assert off == TOT

    order = np.lexsort((src_pid, key))
    sk = key[order]
    bucket_start = np.searchsorted(sk, np.arange(c.NC * c.NT * c.NCHUNK), "left")
    rank = np.zeros(c.E, np.int64)
    rank[order] = np.arange(c.E) - bucket_start[sk]

    slot = base[tile_of, chunk_of] + rank
    gidx_f = np.zeros((c.NC, TOT), np.int16)
    gdst_f = np.full((c.NC, TOT), -1.0, F32)
    oh_f = np.zeros((c.NC, 18, TOT), BF16)
    gidx_f[e_core, slot] = (src_pid - chunk_of * c.CH).astype(np.int16)
    gdst_f[e_core, slot] = (dst_loc - tile_of * 128).astype(F32)
    rows = np.arange(c.E)
    for f in range(3):
        oh_f[e_core, f * c.BOND_V + ex[rows, f], slot] = 1.0

    # atom one-hot: AT_CH chunks of 128 classes; OH900T [AT_CH*128, NPAD]
    AT_ROWS = ((9 * c.ATOM_V + 127) // 128) * 128
    AT_CH = AT_ROWS // 128
    oh900 = np.zeros((c.NC, AT_CH, 128, c.NPAD), BF16)
    for core in range(c.NC):
        xl = x[core * c.NLOC:(core + 1) * c.NLOC]
        cls = (xl + (np.arange(9) * c.ATOM_V)[None, :])   # [NLOC, 9]
        nn_ = np.repeat(np.arange(c.NLOC), 9)
        cf = cls.reshape(-1)
        oh900[core, cf // 128, cf % 128, nn_] = 1.0

    ngf = np.full((c.NC, 128, c.NT), -1.0, F32)
    for core in range(c.NC):
        loc = np.full(c.NPAD, -1.0, F32)
        loc[:c.NLOC] = node_graph[core * c.NLOC:(core + 1) * c.NLOC]
        ngf[core] = loc.reshape(c.NT, 128).T
    cnt = np.bincount(node_graph, minlength=c.G).astype(F32)
    invc_t = (1.0 / np.maximum(cnt, 1.0)).reshape(c.NGT, 128).T.astype(F32).copy()

    t_atom = np.zeros((AT_ROWS, c.D), BF16)
    t_atom[:9 * c.ATOM_V] = atom_emb.reshape(9 * c.ATOM_V, c.D).astype(BF16)
    t_atom = t_atom.reshape(AT_CH, 128, c.D)
    t_bond = np.transpose(bond_emb.reshape(c.L, 18, c.D), (1, 0, 2)).astype(BF16)
    w1s = np.transpose(W1 * (1.0 + eps)[:, None, None], (1, 0, 2)).astype(BF16)
    w1 = np.transpose(W1, (1, 0, 2)).astype(BF16)
    w2 = np.transpose(W2.reshape(c.L, 2, 128, c.D), (2, 0, 1, 3)).astype(BF16)
    # [128, L*12]: col l*12 + {0,1}:g1 {2,3}:be1 {4,5}:unused {6}:b2
    #              {8}:bn_g {10}:bn_b
    bnp = np.zeros((128, c.L * 12), F32)
    for l in range(c.L):
        bnp[:, l * 12 + 0:l * 12 + 2] = g1[l].reshape(2, 128).T
        bnp[:, l * 12 + 2:l * 12 + 4] = be1[l].reshape(2, 128).T
        bnp[:, l * 12 + 6] = b2[l]
        if l < c.L - 1:
            bnp[:, l * 12 + 8] = bn_g[l]
            bnp[:, l * 12 + 10] = bn_b[l]

    ident_bf = np.eye(128, dtype=BF16)
    ident_f32 = np.eye(128, dtype=F32)
    iota_bf = np.tile(np.arange(128, dtype=F32)[None, :], (128, 1)).astype(BF16)
    iota_big = np.tile(np.arange(c.G, dtype=F32)[None, :], (128, 1))

    in_maps = []
    for core in range(c.NC):
        in_maps.append(dict(
            gidx=_wrap16(gidx_f[core]),
            gdst=gdst_f[core].reshape(TOT // 128, 128).T.astype(BF16).copy(),
            goh=np.ascontiguousarray(oh_f[core]),
            oh900=oh900[core],
            ngf=ngf[core],
            t_atom=t_atom, t_bond=t_bond, w1s=w1s, w1=w1, w2=w2, bnp=bnp,
            invc=invc_t, outw=outW.astype(F32),
            outb=outb.reshape(c.OUT, 1).astype(F32),
            ident_bf=ident_bf, ident_f32=ident_f32, iota_bf=iota_bf,
            iota_big=iota_big,
        ))
    meta = dict(K_tc=K_tc, KT=KT, TOT=TOT, AT_ROWS=AT_ROWS, AT_CH=AT_CH)
    return in_maps, meta


# -------------------------------------------------------------- program -----


def build_program(cfg, meta, debug_taps=False):
    c = cfg
    K_tc, KT, TOT, AT_ROWS = meta["K_tc"], meta["KT"], meta["TOT"], meta["AT_ROWS"]
    AT_CH = meta["AT_CH"]
    dt = mybir.dt
    AF = mybir.ActivationFunctionType
    OP = mybir.AluOpType
    RG = [list(range(c.NC))]
    GBCOL = max(9 * c.GTA, c.GT * KT)    # gather buffer columns (shared tag)

    nc = bacc.Bacc("TRN2", target_bir_lowering=False, debug=False,
                   num_devices=c.NC)

    def din(name, shape, d):
        return nc.dram_tensor(name, shape, d, kind="ExternalInput")

    gidx = din("gidx", [128, TOT // 16], dt.int16)
    gdst = din("gdst", [128, TOT // 128], dt.bfloat16)
    goh = din("goh", [18, TOT], dt.bfloat16)
    oh900_d = din("oh900", [AT_CH, 128, c.NPAD], dt.bfloat16)
    ngf_d = din("ngf", [128, c.NT], dt.float32)
    iota_big_d = din("iota_big", [128, c.G], dt.float32)
    t_atom = din("t_atom", [AT_CH, 128, c.D], dt.bfloat16)
    t_bond = din("t_bond", [18, c.L, c.D], dt.bfloat16)
    w1s_d = din("w1s", [c.D, c.L, 2 * c.D], dt.bfloat16)
    w1_d = din("w1", [c.D, c.L, 2 * c.D], dt.bfloat16)
    w2_d = din("w2", [128, c.L, 2, c.D], dt.bfloat16)
    bnp_d = din("bnp", [128, c.L * 12], dt.float32)
    invc_d = din("invc", [128, c.NGT], dt.float32)
    outw_d = din("outw", [c.D, c.OUT], dt.float32)
    outb_d = din("outb", [c.OUT, 1], dt.float32)
    ident_bf_d = din("ident_bf", [128, 128], dt.bfloat16)
    ident_f32_d = din("ident_f32", [128, 128], dt.float32)
    iota_bf_d = din("iota_bf", [128, 128], dt.bfloat16)
    out_d = nc.dram_tensor("out", [c.OUT, c.G], dt.float32, kind="ExternalOutput")

    h_shard = nc.dram_tensor("h_shard", [c.NPAD, c.D], dt.float32)
    h_full = [nc.dram_tensor(f"h_full_{l}", [c.NPT, c.D], dt.float32,
                             addr_space="Shared") for l in range(c.L)]
    st_in = [nc.dram_tensor(f"st_in_{k}", [128, 4], dt.float32)
             for k in range(2 * c.L)]
    st_out = [nc.dram_tensor(f"st_out_{k}", [128, 4], dt.float32,
                             addr_space="Shared") for k in range(2 * c.L)]
    dbg = {}
    if debug_taps:
        for l in range(c.L):
            dbg[f"dbg_h{l}"] = nc.dram_tensor(f"dbg_h{l}", [c.NPT, c.D],
                                              dt.float32, kind="ExternalOutput")
        dbg["dbg_z1"] = nc.dram_tensor("dbg_z1", [2, 128, c.NPAD], dt.bfloat16,
                                       kind="ExternalOutput")
        dbg["dbg_agg"] = nc.dram_tensor("dbg_agg", [128, c.NPAD], dt.bfloat16,
                                        kind="ExternalOutput")
        dbg["dbg_pool"] = nc.dram_tensor("dbg_pool", [c.G, c.D], dt.float32,
                                         kind="ExternalOutput")
    pooled_part2 = nc.dram_tensor("pooled_part2", [c.G, c.D], dt.float32)
    pooled_full = nc.dram_tensor("pooled_full", [c.G, c.D], dt.float32,
                                 addr_space="Shared")

    NREC = 1.0 / float(c.N)

    with tile.TileContext(nc) as tc:
        nc.gpsimd.load_library(library_config.mlp)
        import contextlib
        with contextlib.ExitStack() as ctx:
            P = lambda **kw: ctx.enter_context(tc.tile_pool(**kw))
            consts = P(name="consts", bufs=1)
            hTp = P(name="hTp", bufs=1)
            z1p_ = P(name="z1p", bufs=1)
            gat = P(name="gat", bufs=3)
            str3 = P(name="str3", bufs=3)
            small = P(name="small", bufs=3)
            pool2 = P(name="pool2", bufs=2)
            aggp = P(name="aggp", bufs=max(c.GT, 2) + 2)
            statp = P(name="statp", bufs=2)
            psum2 = P(name="psum2", bufs=3, space="PSUM")
            psum1 = P(name="psum1", bufs=1, space="PSUM")

            # ---------------- constants / params ----------------
            ident_bf = consts.tile([128, 128], dt.bfloat16)
            nc.sync.dma_start(ident_bf[:], ident_bf_d[:])
            ident_f32 = consts.tile([128, 128], dt.float32)
            nc.sync.dma_start(ident_f32[:], ident_f32_d[:])
            iota_bf = consts.tile([128, 128], dt.bfloat16)
            nc.sync.dma_start(iota_bf[:], iota_bf_d[:])
            bond_sb = consts.tile([18, c.L, c.D], dt.bfloat16)
            nc.sync.dma_start(bond_sb[:], t_bond[:])
            w1s_sb = consts.tile([128, c.L, 2 * c.D], dt.bfloat16)
            nc.sync.dma_start(w1s_sb[:], w1s_d[:])
            w1_sb = consts.tile([128, c.L, 2 * c.D], dt.bfloat16)
            nc.sync.dma_start(w1_sb[:], w1_d[:])
            w2_sb = consts.tile([128, c.L, 2, c.D], dt.bfloat16)
            nc.sync.dma_start(w2_sb[:], w2_d[:])
            bnp_sb = consts.tile([128, c.L * 12], dt.float32)
            nc.sync.dma_start(bnp_sb[:], bnp_d[:])
            invc_sb = consts.tile([128, c.NGT], dt.float32)
            nc.sync.dma_start(invc_sb[:], invc_d[:])
            outw_sb = consts.tile([128, c.OUT], dt.float32)
            nc.sync.dma_start(outw_sb[:], outw_d[:])
            outb_sb = consts.tile([c.OUT, 1], dt.float32)
            nc.sync.dma_start(outb_sb[:], outb_d[:])
            epsb = consts.tile([128, 1], dt.float32)
            nc.vector.memset(epsb[:], float(c.BN_EPS))
            ngf_sb = consts.tile([128, c.NT], dt.float32)
            nc.sync.dma_start(ngf_sb[:], ngf_d[:])
            iota_big = consts.tile([128, c.G], dt.float32)
            nc.sync.dma_start(iota_big[:], iota_big_d[:])

            evac_flip = [0]

            def evac_relu(dst_ap, src_ap):
                if evac_flip[0] % 2 == 0:
                    nc.vector.tensor_scalar(dst_ap, src_ap, 0.0, None, OP.max)
                else:
                    nc.scalar.activation(dst_ap, src_ap, AF.Relu)
                evac_flip[0] += 1

            def evac_copy(dst_ap, src_ap):
                if evac_flip[0] % 2 == 0:
                    nc.vector.tensor_copy(dst_ap, src_ap)
                else:
                    nc.scalar.activation(dst_ap, src_ap, AF.Copy)
                evac_flip[0] += 1

            # ---------------- atom encoder (one-hot matmuls) ----------------
            atom_sb = consts.tile([128, AT_CH, c.D], dt.bfloat16)
            nc.sync.dma_start(atom_sb[:], t_atom[:].rearrange("a p d -> p a d"))
            hT_cur = hTp.tile([128, c.NPAD], dt.bfloat16, tag="hT")
            for (t0, gt) in c.groups:
                nn_ = gt * 128
                oha = gat.tile([128, AT_CH, c.GT * 128], dt.bfloat16, tag="gb")
                nc.sync.dma_start(
                    oha[:, :, :nn_],
                    oh900_d[:, :, t0 * 128:(t0 + gt) * 128].rearrange(
                        "a p d -> p a d"))
                # h0T feature-major: [128f, nn] = sum_ch A_ch.T @ OH_ch
                for blk in range(0, nn_, 512):
                    bw = min(512, nn_ - blk)
                    h0p = psum2.tile([128, 512], dt.float32, tag="work")
                    for ch_ in range(AT_CH):
                        nc.tensor.matmul(
                            h0p[:, :bw], lhsT=atom_sb[:, ch_, :],
                            rhs=oha[:, ch_, blk:blk + bw],
                            start=(ch_ == 0), stop=(ch_ == AT_CH - 1))
                    evac_copy(hT_cur[:, t0 * 128 + blk:t0 * 128 + blk + bw],
                              h0p[:, :bw])
                # node-major via transposes for the shard write
                hng = small.tile([128, c.GT * 128], dt.float32, tag="hng")
                for tl in range(gt):
                    t = t0 + tl
                    tp = psum2.tile([128, 128], dt.bfloat16, tag="work")
                    nc.tensor.transpose(tp[:], hT_cur[:, t * 128:(t + 1) * 128],
                                        ident_bf[:])
                    evac_copy(hng[:, tl * 128:(tl + 1) * 128], tp[:])
                nc.sync.dma_start(
                    h_shard[t0 * 128:(t0 + gt) * 128, :].rearrange(
                        "(a p) d -> p a d", p=128),
                    hng[:, :gt * 128].rearrange("p (a d) -> p a d", a=gt))
            nc.gpsimd.collective_compute(
                "AllGather", OP.bypass, replica_groups=RG,
                ins=[h_shard[:]], outs=[h_full[0][:]])

            # ---------------- layers ----------------
            for l in range(c.L):
                last = (l == c.L - 1)
                if debug_taps:
                    for blk in range(0, c.NPT, 128 * 64):
                        nrow = min(128 * 64, c.NPT - blk)
                        na = nrow // 128
                        dbt = small.tile([128, 64, c.D], dt.float32, tag="dbt")
                        nc.sync.dma_start(
                            dbt[:, :na, :],
                            h_full[l][blk:blk + nrow, :].rearrange(
                                "(a p) d -> p a d", p=128))
                        nc.sync.dma_start(
                            dbg[f"dbg_h{l}"][blk:blk + nrow, :].rearrange(
                                "(a p) d -> p a d", p=128),
                            dbt[:, :na, :])
                B_l = bond_sb[:, l, :]
                sum_a = statp.tile([128, c.NT], dt.float32, tag="sa")
                sum_b = statp.tile([128, c.NT], dt.float32, tag="sb")
                sq_a = statp.tile([128, c.NT], dt.float32, tag="qa")
                sq_b = statp.tile([128, c.NT], dt.float32, tag="qb")
                z1a = z1p_.tile([128, c.NPAD], dt.bfloat16, tag="z1a")
                z1b = z1p_.tile([128, c.NPAD], dt.bfloat16, tag="z1b")

                for (t0, gt) in c.groups:
                    nst = gt * KT
                    agg_t = []
                    for _ai in range(gt):
                        agg_i = psum1.tile([128, 128], dt.float32,
                                           tag=f"agg{_ai}")
                        agg_t.append(agg_i)

                    def aggv(tl):
                        return agg_t[tl][:]

                    for ch in range(c.NCHUNK):
                        call_base = t0 * c.NCHUNK * K_tc + ch * gt * K_tc
                        S = gt * K_tc
                        gi = str3.tile([128, c.GT * K_tc // 16], dt.int16, tag="gi")
                        nc.sync.dma_start(gi[:, :S // 16],
                                          gidx[:, call_base // 16:(call_base + S) // 16])
                        gb = gat.tile([128, GBCOL, 128], dt.float32, tag="gb")
                        nc.gpsimd.dma_gather(
                            gb[:, :nst, :], h_full[l][ch * c.CH:(ch + 1) * c.CH, :],
                            gi[:, :S // 16], S, S, c.D, elem_step=c.D,
                            single_packet=False)
                        oh = str3.tile([18, c.GT * K_tc], dt.bfloat16, tag="oh")
                        nc.sync.dma_start(oh[:, :S], goh[:, call_base:call_base + S])
                        db = str3.tile([128, c.GT * KT], dt.bfloat16, tag="db")
                        nc.sync.dma_start(db[:, :nst],
                                          gdst[:, call_base // 128:(call_base + S) // 128])
                        for b0 in range(0, nst, 4):
                            nb = min(4, nst - b0)
                            mp = psum2.tile([128, 512], dt.float32, tag="work")
                            nc.tensor.matmul(
                                mp[:, :nb * 128], lhsT=ident_f32[:],
                                rhs=gb[:, b0:b0 + nb, :].rearrange(
                                    "p a d -> p (a d)"),
                                start=True, stop=False)
                            for j in range(nb):
                                s = b0 + j
                                q = mp[:, j * 128:(j + 1) * 128]
                                nc.tensor.matmul(q, lhsT=oh[:, s * 128:(s + 1) * 128],
                                                 rhs=B_l, start=False,
                                                 stop=(j == nb - 1))
                            pb = small.tile([128, 512], dt.bfloat16, tag="pb")
                            d_ap = db[:, b0:b0 + nb]
                            in0 = bass.AP(tensor=d_ap.tensor, offset=d_ap.offset,
                                          ap=[list(d_ap.ap[0]), list(d_ap.ap[1]),
                                              [0, 128]])
                            i_ap = iota_bf[:]
                            in1 = bass.AP(tensor=i_ap.tensor, offset=i_ap.offset,
                                          ap=[list(i_ap.ap[0]), [0, nb],
                                              list(i_ap.ap[1])])
                            pb3 = pb[:, :nb * 128].rearrange("p (a d) -> p a d", a=nb)
                            nc.vector.tensor_tensor(out=pb3, in0=in0, in1=in1,
                                                    op=OP.is_equal)
                            ms = small.tile([128, 512], dt.bfloat16, tag="ms")
                            evac_relu(ms[:, :nb * 128], mp[:, :nb * 128])
                            for j in range(nb):
                                s = b0 + j
                                tl, k = s // KT, s % KT
                                nc.tensor.matmul(
                                    aggv(tl), lhsT=ms[:, j * 128:(j + 1) * 128],
                                    rhs=pb[:, j * 128:(j + 1) * 128],
                                    start=(ch == 0 and k == 0),
                                    stop=(ch == c.NCHUNK - 1 and k == KT - 1))
                    # z1 for this group's tiles
                    for tl in range(gt):
                        t = t0 + tl
                        ags = aggp.tile([128, 128], dt.bfloat16, tag="ags")
                        evac_copy(ags[:], aggv(tl))
                        if debug_taps and l == 0:
                            nc.sync.dma_start(
                                dbg["dbg_agg"][:, t * 128:(t + 1) * 128], ags[:])
                        zp = psum2.tile([128, 256], dt.float32, tag="work")
                        nsl = slice(t * 128, (t + 1) * 128)
                        for h in range(2):
                            q = zp[:, h * 128:(h + 1) * 128]
                            nc.tensor.matmul(q,
                                             lhsT=w1s_sb[:, l, h * 128:(h + 1) * 128],
                                             rhs=hT_cur[:, nsl], start=True, stop=False)
                            nc.tensor.matmul(q,
                                             lhsT=w1_sb[:, l, h * 128:(h + 1) * 128],
                                             rhs=ags[:], start=False, stop=True)
                        nc.scalar.activation(z1a[:, nsl], zp[:, 0:128], AF.Copy,
                                             accum_out=sum_a[:, t:t + 1])
                        nc.scalar.activation(z1b[:, nsl], zp[:, 128:256], AF.Copy,
                                             accum_out=sum_b[:, t:t + 1])
                        sqs = small.tile([128, 128], dt.bfloat16, tag="sqs")
                        nc.scalar.activation(sqs[:], z1a[:, nsl], AF.Square,
                                             accum_out=sq_a[:, t:t + 1])
                        sqs2 = small.tile([128, 128], dt.bfloat16, tag="sqs")
                        nc.scalar.activation(sqs2[:], z1b[:, nsl], AF.Square,
                                             accum_out=sq_b[:, t:t + 1])

                # ---- BN1 barrier (b1 cancels under BN) ----
                stp = statp.tile([128, 4], dt.float32, tag="stp")
                nc.vector.tensor_reduce(stp[:, 0:1], sum_a[:],
                                        mybir.AxisListType.X, OP.add)
                nc.vector.tensor_reduce(stp[:, 1:2], sum_b[:],
                                        mybir.AxisListType.X, OP.add)
                nc.vector.tensor_reduce(stp[:, 2:3], sq_a[:],
                                        mybir.AxisListType.X, OP.add)
                nc.vector.tensor_reduce(stp[:, 3:4], sq_b[:],
                                        mybir.AxisListType.X, OP.add)
                nc.sync.dma_start(st_in[2 * l][:], stp[:])
                nc.gpsimd.collective_compute(
                    "AllReduce", OP.add, replica_groups=RG,
                    ins=[st_in[2 * l][:]], outs=[st_out[2 * l][:]])
                st = statp.tile([128, 4], dt.float32, tag="st")
                nc.sync.dma_start(st[:], st_out[2 * l][:])
                AB = statp.tile([128, 8], dt.float32, tag="AB")
                mu, msq, rs, A1 = AB[:, 0:2], AB[:, 2:4], AB[:, 4:6], AB[:, 6:8]
                nc.vector.tensor_scalar(mu, st[:, 0:2], NREC, None, OP.mult)
                nc.vector.tensor_scalar(msq, st[:, 2:4], NREC, None, OP.mult)
                tmp = statp.tile([128, 2], dt.float32, tag="tmp")
                nc.vector.tensor_tensor(tmp[:], mu, mu, OP.mult)
                nc.vector.tensor_tensor(rs, msq, tmp[:], OP.subtract)
                nc.scalar.activation(rs, rs, AF.Sqrt, bias=epsb[:])
                nc.vector.reciprocal(rs, rs)
                g1h = bnp_sb[:, l * 12 + 0:l * 12 + 2]
                be1h = bnp_sb[:, l * 12 + 2:l * 12 + 4]
                nc.vector.tensor_tensor(A1, rs, g1h, OP.mult)
                nc.vector.tensor_tensor(tmp[:], mu, A1, OP.mult)
                B1v = statp.tile([128, 2], dt.float32, tag="B1v")
                nc.vector.tensor_tensor(B1v[:], be1h, tmp[:], OP.subtract)

                for (t0, gt) in c.groups:
                    sl = slice(t0 * 128, (t0 + gt) * 128)
                    nc.scalar.activation(z1a[:, sl], z1a[:, sl], AF.Relu,
                                         bias=B1v[:, 0:1], scale=AB[:, 6:7])
                    nc.scalar.activation(z1b[:, sl], z1b[:, sl], AF.Relu,
                                         bias=B1v[:, 1:2], scale=AB[:, 7:8])
                if c.NLOC < c.NPAD:
                    nc.vector.memset(z1a[:, c.NLOC:c.NPAD], 0.0)
                    nc.vector.memset(z1b[:, c.NLOC:c.NPAD], 0.0)
                if debug_taps and l == 0:
                    nc.sync.dma_start(dbg["dbg_z1"][0], z1a[:])
                    nc.sync.dma_start(dbg["dbg_z1"][1], z1b[:])

                # ---- W2 (+BN2 | +b2 & pooling) ----
                if not last:
                    sum2 = statp.tile([128, c.NT], dt.float32, tag="sa")
                    sq2 = statp.tile([128, c.NT], dt.float32, tag="qa")
                    h_nxt = hTp.tile([128, c.NPAD], dt.bfloat16, tag="hT")
                    for t in range(c.NT):
                        nsl = slice(t * 128, (t + 1) * 128)
                        zp2 = psum2.tile([128, 256], dt.float32, tag="work")
                        q = zp2[:, 0:128]
                        nc.tensor.matmul(q, lhsT=w2_sb[:, l, 0, :], rhs=z1a[:, nsl],
                                         start=True, stop=False)
                        nc.tensor.matmul(q, lhsT=w2_sb[:, l, 1, :], rhs=z1b[:, nsl],
                                         start=False, stop=True)
                        nc.scalar.activation(h_nxt[:, nsl], q, AF.Copy,
                                             accum_out=sum2[:, t:t + 1])
                        sqs3 = small.tile([128, 128], dt.bfloat16, tag="sqs")
                        nc.scalar.activation(sqs3[:], h_nxt[:, nsl], AF.Square,
                                             accum_out=sq2[:, t:t + 1])
                    # BN2 barrier (b2 cancels under BN)
                    stp2 = statp.tile([128, 4], dt.float32, tag="stp")
                    nc.vector.memset(stp2[:], 0.0)
                    nc.vector.tensor_reduce(stp2[:, 0:1], sum2[:],
                                            mybir.AxisListType.X, OP.add)
                    nc.vector.tensor_reduce(stp2[:, 1:2], sq2[:],
                                            mybir.AxisListType.X, OP.add)
                    nc.sync.dma_start(st_in[2 * l + 1][:], stp2[:])
                    nc.gpsimd.collective_compute(
                        "AllReduce", OP.add, replica_groups=RG,
                        ins=[st_in[2 * l + 1][:]], outs=[st_out[2 * l + 1][:]])
                    st2 = statp.tile([128, 4], dt.float32, tag="st")
                    nc.sync.dma_start(st2[:], st_out[2 * l + 1][:])
                    AB2 = statp.tile([128, 4], dt.float32, tag="AB2")
                    mu2, rs2, A2, B2 = (AB2[:, 0:1], AB2[:, 1:2],
                                        AB2[:, 2:3], AB2[:, 3:4])
                    nc.vector.tensor_scalar(mu2, st2[:, 0:1], NREC, None, OP.mult)
                    nc.vector.tensor_scalar(rs2, st2[:, 1:2], NREC, None, OP.mult)
                    t2 = statp.tile([128, 1], dt.float32, tag="t2")
                    nc.vector.tensor_tensor(t2[:], mu2, mu2, OP.mult)
                    nc.vector.tensor_tensor(rs2, rs2, t2[:], OP.subtract)
                    nc.scalar.activation(rs2, rs2, AF.Sqrt, bias=epsb[:])
                    nc.vector.reciprocal(rs2, rs2)
                    gng = bnp_sb[:, l * 12 + 8:l * 12 + 9]
                    gnb = bnp_sb[:, l * 12 + 10:l * 12 + 11]
                    nc.vector.tensor_tensor(A2, rs2, gng, OP.mult)
                    nc.vector.tensor_tensor(t2[:], mu2, A2, OP.mult)
                    nc.vector.tensor_tensor(B2, gnb, t2[:], OP.subtract)
                    for (t0, gt) in c.groups:
                        sl = slice(t0 * 128, (t0 + gt) * 128)
                        nc.scalar.activation(h_nxt[:, sl], h_nxt[:, sl], AF.Relu,
                                             bias=B2, scale=A2)
                    if c.NLOC < c.NPAD:
                        nc.vector.memset(h_nxt[:, c.NLOC:c.NPAD], 0.0)
                    for (t0, gt) in c.groups:
                        hng2 = small.tile([128, c.GT * 128], dt.float32, tag="hng")
                        for tl in range(gt):
                            t = t0 + tl
                            tpb = psum2.tile([128, 128], dt.bfloat16, tag="work")
                            nc.tensor.transpose(tpb[:],
                                                h_nxt[:, t * 128:(t + 1) * 128],
                                                ident_bf[:])
                            evac_copy(hng2[:, tl * 128:(tl + 1) * 128], tpb[:])
                        nc.sync.dma_start(
                            h_shard[t0 * 128:(t0 + gt) * 128, :].rearrange(
                                "(a p) d -> p a d", p=128),
                            hng2[:, :gt * 128].rearrange("p (a d) -> p a d", a=gt))
                    nc.gpsimd.collective_compute(
                        "AllGather", OP.bypass, replica_groups=RG,
                        ins=[h_shard[:]], outs=[h_full[l + 1][:]])
                    hT_cur = h_nxt
                else:
                    # last layer: h3 = z2 + b2 per tile -> transpose to
                    # node-major bf16; pooling via one-hot graph matmuls
                    b2v = bnp_sb[:, l * 12 + 6:l * 12 + 7]
                    h3n = hTp.tile([128, c.NPAD], dt.bfloat16, tag="hT")
                    for t in range(c.NT):
                        nsl = slice(t * 128, (t + 1) * 128)
                        zp2 = psum2.tile([128, 256], dt.float32, tag="work")
                        q = zp2[:, 0:128]
                        nc.tensor.matmul(q, lhsT=w2_sb[:, l, 0, :],
                                         rhs=z1a[:, nsl], start=True, stop=False)
                        nc.tensor.matmul(q, lhsT=w2_sb[:, l, 1, :],
                                         rhs=z1b[:, nsl], start=False, stop=True)
                        h3t = pool2.tile([128, 128], dt.bfloat16, tag="h3t")
                        nc.scalar.activation(h3t[:], q, AF.Identity, bias=b2v)
                        tpf = psum2.tile([128, 128], dt.bfloat16, tag="work")
                        nc.tensor.transpose(tpf[:], h3t[:], ident_bf[:])
                        evac_copy(h3n[:, nsl], tpf[:])
                    # two passes over graph chunks (5 per pass, agg psum banks)
                    for p0 in range(0, c.NGT, 5):
                        pn = min(5, c.NGT - p0)
                        pacc = []
                        for _pi in range(pn):
                            pacc_i = psum1.tile([128, 128], dt.float32,
                                                tag=f"agg{_pi}")
                            pacc.append(pacc_i)
                        for t in range(c.NT):
                            nsl = slice(t * 128, (t + 1) * 128)
                            pgt = pool2.tile([128, 5, 128], dt.bfloat16, tag="pgt")
                            nc.vector.tensor_scalar(
                                pgt[:, :pn, :],
                                iota_big[:, p0 * 128:(p0 + pn) * 128].rearrange(
                                    "p (a d) -> p a d", a=pn),
                                ngf_sb[:, t:t + 1], None, OP.is_equal)
                            for pi in range(pn):
                                nc.tensor.matmul(
                                    pacc[pi][:], lhsT=pgt[:, pi, :],
                                    rhs=h3n[:, nsl],
                                    start=(t == 0), stop=(t == c.NT - 1))
                        for pi in range(pn):
                            pev = pool2.tile([128, 128], dt.float32, tag="pev")
                            evac_copy(pev[:], pacc[pi][:])
                            nc.sync.dma_start(
                                pooled_part2[(p0 + pi) * 128:(p0 + pi + 1) * 128, :],
                                pev[:])
                    nc.gpsimd.collective_compute(
                        "AllReduce", OP.add, replica_groups=RG,
                        ins=[pooled_part2[:]], outs=[pooled_full[:]])
                    pooledT = consts.tile([128, c.G], dt.float32)
                    for gi_ in range(c.NGT):
                        pl = small.tile([128, 128], dt.float32, tag="pl")
                        nc.sync.dma_start(pl[:],
                                          pooled_full[gi_ * 128:(gi_ + 1) * 128, :])
                        nc.vector.tensor_scalar(pl[:], pl[:],
                                                invc_sb[:, gi_:gi_ + 1], None,
                                                OP.mult)
                        tpf2 = psum2.tile([128, 128], dt.float32, tag="work")
                        nc.tensor.transpose(tpf2[:], pl[:], ident_f32[:])
                        evac_copy(pooledT[:, gi_ * 128:(gi_ + 1) * 128], tpf2[:])
                    ob = consts.tile([128, c.G], dt.float32)
                    for k0 in range(0, c.G, 512):
                        kn = min(512, c.G - k0)
                        op_ = psum2.tile([128, 512], dt.float32, tag="work")
                        nc.tensor.matmul(op_[:, :kn], lhsT=outw_sb[:],
                                         rhs=pooledT[:, k0:k0 + kn],
                                         start=True, stop=True)
                        nc.scalar.activation(ob[:, k0:k0 + kn], op_[:, :kn],
                                             AF.Identity, bias=outb_sb[:])
                    nc.sync.dma_start(out_d[:], ob[:])

    nc.compile()
    return nc


# ----------------------------------------------------------------- runner ----

_CACHE = {}


def _get_program(cfg, meta):
    key = (cfg.N, cfg.E, cfg.G, meta["K_tc"])
    if key not in _CACHE:
        _CACHE[key] = build_program(cfg, meta)
    return _CACHE[key]


def run(inputs, cfg=None, trace=False):
    cfg = cfg or Cfg()
    in_maps, meta = preprocess(cfg, inputs)
    nc = _get_program(cfg, meta)
    res = run_bass_kernel_spmd(nc, in_maps, list(range(cfg.NC)), trace=trace)
    out = np.asarray(res.results[0]["out"], np.float32).T.copy()
    return out, res


def kernel(**inputs):
    out, _ = run(inputs)
    return out

